# revision 19
# baseline (speedup 1.0000x reference)
"""Trainium2 Bass kernel for nn_Attention_82660940579436 (v2).

Computation (see reference):
    q     = mean_s(hidden @ Wq.T + bq)            [B, H]
    key   = tanh(hidden @ Wk.T + bk)              [S, B, H]
    score = einsum('bsh,bh->bs', key, q) + mask   [B, S]
    out   = softmax(score) @ key                  [B, H]

Key observations driving this version:
  * Tokens with s >= lengths[b] get softmax weight exactly 0, so keys /
    scores / weighted sums are only needed for s < lengths[b] (a PREFIX of
    each batch's tokens).  Only the q-mean needs every token.
  * The host can pre-transpose + pre-cast hidden to bf16 "hT" layout
    [jc, j, tok] so the device does ZERO transposes: the z matmul consumes
    hT chunks as the stationary operand directly from DMA.
  * Batches are assigned to (core, slot) so that each slot's max length
    (over cores) is small: sort lengths desc, slot s takes ranks [8s, 8s+8).
    All cores then run the SAME program shape (z-tile counts per slot are
    global maxima); per-core data (hT, masks) differs.

Device program per core (4 slots x 4096 tokens; z-tiles of 128 tokens):
  Phase A, per 2048-token chunk-group (8 groups, z-rich first):
    - 4 HWDGE DMAs load hT chunks [128 j, 2048 tok] bf16 (one per j-chunk)
    - mean: per chunk, fold-tree (DVE) or Copy+accum (ACT) -> csum [128,1];
      m[j, (jc,g)] = csum_h0 + csum_h1 (bf16)
    - per z-tile: PE bias matmul (ones x bk) + 4 z matmuls (hT chunk
      stationary, WkT moving) -> PSUM [128 tok, 512]; ACT tanh -> keys bf16
  q block (emitted mid z-stream so PE reaches it right as the mean lands):
    q = m @ (WqT/S) (PE, bf16) ; q += bq (DVE, reads PSUM) ;
    qrep_g = sel_g.T @ q (PE) -> SBUF bf16 [128, 512] per slot
  Phase B, per z-tile:
    prod = keys[t] * qrep_slot   (DVE 2x / Pool split)
    score = rowsum(prod)         (DVE fold-tree / ACT accum split)
    e = exp(score + mask)        (ACT; mask -60 for invalid tokens)
    ei = ind_slot * e            (DVE tensor_scalar [128,4] bf16)
    numer += ei.T @ keys[t] ; den += ei.T @ ones   (PE, PSUM accumulate)
  out = numer / den -> DMA

Cost-model notes (TimelineSim/InstructionCostModel is the graded metric):
  matmul = out_free x 0.4167ns (bf16, warm); DMA = desc/16 x elem/22.5 (2x
  penalty below 512B runs -- hence 2048-token bf16 chunk rows); DVE
  TensorTensor bf16 SBUF = 2x mode; TensorReduce = 1x; ACT = 1/cycle
  + ~185ns init, accum_out +187ns.  fp8 DoubleRow would halve PE but
  measures 3.9e-2 rel err (> 2e-2 gate) -- rejected.
"""

import sys

import numpy as np

if "/opt/trn_rl_repo" not in sys.path:
    sys.path.append("/opt/trn_rl_repo")

import ml_dtypes  # noqa: E402

FP8NP = ml_dtypes.float8_e4m3fn

import concourse.bacc as bacc  # noqa: E402
import concourse.mybir as mybir  # noqa: E402
import concourse.tile as tile  # noqa: E402
from concourse.bass_utils import run_bass_kernel_spmd  # noqa: E402

S, B, H = 4096, 32, 512
NCORES = 8
SLOTS = 4  # batches per core
SLOT_TOK = S  # tokens per slot
CHUNK = 2048  # tokens per DMA chunk (4KB bf16 rows: no <512B DMA penalty)
JC = H // 128  # 4 j-chunks
TOK_CORE = SLOTS * SLOT_TOK

F32 = mybir.dt.float32
BF16 = mybir.dt.bfloat16
FP8 = mybir.dt.float8e4
AF = mybir.ActivationFunctionType
ALU = mybir.AluOpType
BF16NP = ml_dtypes.bfloat16
MASK_NEG = -60.0

# bf16 const pack offsets (elements); WqT/S ships separately (packq) so the
# startup-critical const DMA stays small.
OB_WK = 0  # [128, 4*512] WkT chunks
OB_SEL = 2048  # [4, 4*128] qrep selectors
OB_IND = 2560  # [128, 4*4] slot indicators
OB_ONESR = 2576  # [1, 128] ones row
OB_ONESC = 2704  # [128, 1] ones col
OB_BK = 2705  # [1, 512] bk
PB = 3217
# f32 const pack offsets
PF_PAD = 80  # mask columns (>= NZ)
OF_MASK = 0  # [128, PF_PAD]
OF_ZERO = PF_PAD  # [128, 1]
OF_BQ = PF_PAD + 1  # [4, 512] bq rows
PF = PF_PAD + 1 + 512

KNOBS = {
    "zps_bufs": 2,
    "fullz": 4,  # z-groups loaded full via the 2-buf xf pool (serve mean too)
    "q_after": 35,  # emit q block after this many z-tiles (min: first 2 groups)
    "b_catch": 4,  # phase-B tiles advanced per z-tile once past QI
    "b_stagger": 2,  # numer/TSP trail the mul/fold front by this many tiles
    "mul_pool_mod": 2,  # z-tile zi uses Pool mul when zi % mod == mod-1
    "red_act_mod": 3,  # z-tile zi reduces via ACT accum when zi % mod == 1
}

import json as _json
import os as _os

if _os.environ.get("KERNEL_KNOBS"):
    KNOBS.update(_json.loads(_os.environ["KERNEL_KNOBS"]))


def _plan(lengths):
    lens = np.asarray(lengths).astype(np.int64)
    order = np.argsort(-lens, kind="stable")
    batch_of = np.zeros((NCORES, SLOTS), dtype=np.int64)
    for s in range(SLOTS):
        for c in range(NCORES):
            batch_of[c, s] = order[NCORES * s + c]
    K = []
    for s in range(SLOTS):
        mx = int(lens[order[NCORES * s : NCORES * (s + 1)]].max())
        K.append(min(32, -(-mx // 128)))
    groups = []  # (slot, half, nz)
    for s in range(SLOTS):
        for hh in range(2):
            nz = max(0, min(16, K[s] - 16 * hh))
            groups.append((s, hh, nz))
    groups.sort(key=lambda x: (-x[2], x[0], x[1]))
    return batch_of, K, groups


def _build_kernel_body(tc, aps, groups):
    nc = tc.nc
    xh, packb, packf, y = aps["xh"], aps["packb"], aps["packf"], aps["y"]
    NZ = sum(g[2] for g in groups)

    zgroups = [g for g in groups if g[2] > 0]  # z-order (nz desc)
    mgroups = [g for g in groups if g[2] == 0]  # mean-only
    NFULL = min(KNOBS["fullz"], len(zgroups))
    fullz = zgroups[:NFULL]
    trimz = zgroups[NFULL:]
    dls = mgroups + trimz  # groups whose full chunk loads via the dl pool

    from contextlib import ExitStack

    with ExitStack() as ctx:
        consts = ctx.enter_context(tc.tile_pool(name="consts", bufs=1))
        pxf = ctx.enter_context(tc.tile_pool(name="xf", bufs=2))
        pdl = ctx.enter_context(tc.tile_pool(name="dl", bufs=2))
        ptz = ctx.enter_context(tc.tile_pool(name="tz", bufs=1))
        pkeys = ctx.enter_context(tc.tile_pool(name="keys", bufs=max(NZ, 1)))
        pfold = ctx.enter_context(tc.tile_pool(name="fold", bufs=4))
        pascr = ctx.enter_context(tc.tile_pool(name="ascr", bufs=2))
        pprod = ctx.enter_context(tc.tile_pool(name="prod", bufs=4))
        psmall = ctx.enter_context(tc.tile_pool(name="small", bufs=6))
        pacc = ctx.enter_context(tc.tile_pool(name="acc", bufs=1))
        ps_z = ctx.enter_context(
            tc.tile_pool(name="ps_z", bufs=KNOBS["zps_bufs"], space="PSUM")
        )
        ps_q = ctx.enter_context(tc.tile_pool(name="ps_q", bufs=1, space="PSUM"))
        ps_qr = ctx.enter_context(tc.tile_pool(name="ps_qr", bufs=2, space="PSUM"))
        ps_acc = ctx.enter_context(tc.tile_pool(name="ps_acc", bufs=1, space="PSUM"))

        cb = consts.tile([128, PB], BF16)
        # small consts (bk/ones/ind/sel) land in ~1us; WK chunks follow
        # interleaved with group 0's loads so the PE starts at ~3us.
        nc.sync.dma_start(cb[:, 2048:PB], packb[:, 2048:PB])
        cf = consts.tile([128, PF], F32)
        cq = consts.tile([128, 2048], BF16)  # WqT/S; DMA deferred
        c8 = consts.tile([1, 1280], FP8)
        nc.sync.dma_start(c8, aps["pack8"])
        ones8_dr = c8[0:1, 0:256].rearrange("p (two f) -> p two f", two=2)
        bk8_dr = c8[0:1, 256:1280].rearrange("p (two f) -> p two f", two=2)

        def wk_sb(c):
            return cb[:, OB_WK + c * 512 : OB_WK + (c + 1) * 512]

        def wq_sb(c):
            return cq[:, c * 512 : (c + 1) * 512]

        def sel_sb(g):
            return cb[0:SLOTS, OB_SEL + g * 128 : OB_SEL + (g + 1) * 128]

        def ind_sb(g):
            return cb[:, OB_IND + g * SLOTS : OB_IND + (g + 1) * SLOTS]

        ones_row = cb[0:1, OB_ONESR : OB_ONESR + 128]
        ones_col = cb[:, OB_ONESC : OB_ONESC + 1]
        bk_row = cb[0:1, OB_BK : OB_BK + 512]
        mask_sb = cf[:, OF_MASK : OF_MASK + PF_PAD]
        zero_sb = cf[:, OF_ZERO : OF_ZERO + 1]
        bq_sb = cf[0:SLOTS, OF_BQ : OF_BQ + 512]

        m_sb = pacc.tile([128, SLOTS * JC], BF16)  # col = jc*4 + g
        mparts = pacc.tile([128, 2 * SLOTS * JC], F32)  # col = (jc*4+g)*2 + half

        # ---------------- emission helpers ----------------
        def emit_mean(xt, s, hh):
            """chunk tiles -> csum [128,1] per jc, into mparts."""
            for jc in range(JC):
                dst = mparts[
                    :, (jc * SLOTS + s) * 2 + hh : (jc * SLOTS + s) * 2 + hh + 1
                ]
                f = pfold.tile([128, 1024], BF16, tag="fold")
                nc.vector.tensor_add(f, xt[jc][:, 0:1024], xt[jc][:, 1024:2048])
                nc.vector.tensor_add(f[:, 0:512], f[:, 0:512], f[:, 512:1024])
                nc.vector.tensor_add(f[:, 0:256], f[:, 0:256], f[:, 256:512])
                nc.vector.tensor_add(f[:, 0:128], f[:, 0:128], f[:, 128:256])
                nc.vector.tensor_reduce(
                    dst, f[:, 0:128], axis=mybir.AxisListType.X, op=ALU.add
                )

        def load_full(s, hh):
            base = s * SLOT_TOK + hh * CHUNK
            xt = []
            for jc in range(JC):
                t = pxf.tile([128, CHUNK], BF16, tag=f"xf{jc}")
                nc.sync.dma_start(t, xh[jc, :, base : base + CHUNK])
                xt.append(t)
            emit_mean(xt, s, hh)
            return xt

        def load_dl(s, hh):
            base = s * SLOT_TOK + hh * CHUNK
            xt = []
            for jc in range(JC):
                t = pdl.tile([128, CHUNK], BF16, tag=f"dl{jc}")
                nc.sync.dma_start(t, xh[jc, :, base : base + CHUNK])
                xt.append(t)
            emit_mean(xt, s, hh)

        def load_trim(idx, s, hh, nz):
            base = s * SLOT_TOK + hh * CHUNK
            w = nz * 128
            xt = []
            for jc in range(JC):
                t = ptz.tile([128, w], BF16, tag=f"tz{idx}_{jc}")
                nc.sync.dma_start(t, xh[jc, :, base : base + w])
                xt.append(t)
            return xt

        def emit_madds():
            for col in range(SLOTS * JC):
                nc.vector.tensor_add(
                    m_sb[:, col : col + 1],
                    mparts[:, 2 * col : 2 * col + 1],
                    mparts[:, 2 * col + 1 : 2 * col + 2],
                )

        qreps = []

        def emit_q_block():
            q_ps = ps_q.tile([SLOTS, 512], F32, tag="q")
            for jc in range(JC):
                nc.tensor.matmul(
                    q_ps,
                    m_sb[:, jc * SLOTS : (jc + 1) * SLOTS],
                    wq_sb(jc),
                    start=(jc == 0),
                    stop=(jc == JC - 1),
                )
            q_sbt = pacc.tile([SLOTS, 512], BF16)
            nc.vector.tensor_add(q_sbt, q_ps, bq_sb)
            for g in range(SLOTS):
                qr_ps = ps_qr.tile([128, 512], F32, tag="qr")
                nc.tensor.matmul(qr_ps, sel_sb(g), q_sbt, start=True, stop=True)
                qr = pacc.tile([128, 512], BF16, tag=f"qrep{g}")
                nc.vector.tensor_copy(qr, qr_ps)
                qreps.append(qr)

        # ---------------- phase B emitters (front/back stagger) ----------------
        numer = ps_acc.tile([SLOTS, 512], F32, tag="numer")
        den = ps_acc.tile([SLOTS, 1], F32, tag="den")
        keys = []
        zslot = []
        e_tiles = []

        def emit_front(zi):
            kt = keys[zi]
            s = zslot[zi]
            prod = pprod.tile([128, 512], BF16, tag="prod")
            if zi % KNOBS["mul_pool_mod"] == KNOBS["mul_pool_mod"] - 1:
                nc.gpsimd.tensor_mul(prod, kt, qreps[s])
            else:
                nc.vector.tensor_mul(prod, kt, qreps[s])
            sc = psmall.tile([128, 1], F32, tag="sc")
            if zi % KNOBS["red_act_mod"] == 1:
                scr = pascr.tile([128, 512], BF16, tag="bscr")
                nc.scalar.activation(scr, prod, AF.Copy, accum_out=sc)
            else:
                nc.vector.tensor_add(prod[:, 0:256], prod[:, 0:256], prod[:, 256:512])
                nc.vector.tensor_add(prod[:, 0:128], prod[:, 0:128], prod[:, 128:256])
                nc.vector.tensor_reduce(
                    sc, prod[:, 0:128], axis=mybir.AxisListType.X, op=ALU.add
                )
            e_t = psmall.tile([128, 1], F32, tag="e")
            nc.scalar.activation(e_t, sc, AF.Exp, bias=mask_sb[:, zi : zi + 1])
            e_tiles.append(e_t)

        def emit_back(zi):
            kt = keys[zi]
            ei = psmall.tile([128, SLOTS], BF16, tag="ei")
            nc.vector.tensor_scalar_mul(ei, ind_sb(zslot[zi]), e_tiles[zi])
            nc.tensor.matmul(numer, ei, kt, start=(zi == 0), stop=(zi == NZ - 1))
            nc.tensor.matmul(den, ei, ones_col, start=(zi == 0), stop=(zi == NZ - 1))

        # ---------------- the merged A/B schedule ----------------
        # DMA issue order (SP queue is FIFO): full z-groups interleaved with
        # dl (mean-copy) loads so every mean source has landed by ~40us while
        # the PE never waits for its next z chunk.
        ztile_plan = []  # (xt, local t, slot)

        def plan_group(xt, s, nz):
            for t in range(nz):
                ztile_plan.append((xt, t, s))

        # batch0 inline: WK slices first, then group 0
        for jc in range(JC):
            nc.sync.dma_start(
                cb[:, OB_WK + jc * 512 : OB_WK + (jc + 1) * 512],
                packb[:, OB_WK + jc * 512 : OB_WK + (jc + 1) * 512],
            )
        nst = 0
        xt0 = load_full(*fullz[0][:2])
        nc.sync.dma_start(cf, packf)
        plan_group(xt0, fullz[0][0], fullz[0][2])
        if NFULL > 1:
            xt1 = load_full(*fullz[1][:2])
            plan_group(xt1, fullz[1][0], fullz[1][2])
        for d in dls[0:2]:
            load_dl(d[0], d[1])
        # batch1/2 described as thunks, emitted at group boundaries
        def emit_batch1():
            if NFULL > 2:
                xt = load_full(*fullz[2][:2])
                plan_group(xt, fullz[2][0], fullz[2][2])
            for d in dls[2:4]:
                load_dl(d[0], d[1])

        def emit_batch2():
            if NFULL > 3:
                xt = load_full(*fullz[3][:2])
                plan_group(xt, fullz[3][0], fullz[3][2])
            nc.sync.dma_start(cq, aps["packq"])
            for d in dls[4:]:
                load_dl(d[0], d[1])
            for i, (s, hh, nz) in enumerate(trimz):
                xt = load_trim(i, s, hh, nz)
                plan_group(xt, s, nz)
            emit_madds()

        tiles01 = nst + fullz[0][2] + (fullz[1][2] if NFULL > 1 else 0)
        QI = max(min(KNOBS["q_after"], NZ - 1), min(tiles01 + 1, NZ - 1))
        bnd1 = nst + fullz[0][2]  # after group 0's tiles
        bnd2 = tiles01  # after group 1's tiles

        zi = 0
        fj = 0  # phase B front progress

        def emit_ztile(xt, t, s):
            zp = ps_z.tile([128, 512], F32, tag="z")
            # fp8 DoubleRow rank-2 bias: out = sum_i ones8[:,i,:].T @ bk8[:,i,:]
            # = bk broadcast over tokens, at 0.5 cycles/row (half the bf16 cost)
            nc.tensor.matmul(
                zp,
                ones8_dr,
                bk8_dr,
                start=True,
                stop=False,
                perf_mode=mybir.MatmulPerfMode.DoubleRow,
            )
            for jc in range(JC):
                nc.tensor.matmul(
                    zp,
                    xt[jc][:, t * 128 : (t + 1) * 128],
                    wk_sb(jc),
                    start=False,
                    stop=(jc == JC - 1),
                )
            kt = pkeys.tile([128, 512], BF16, tag="key")
            nc.scalar.activation(kt, zp, AF.Tanh, bias=zero_sb)
            keys.append(kt)
            zslot.append(s)

        while zi < NZ or fj < NZ:
            if zi < len(ztile_plan):
                emit_ztile(*ztile_plan[zi])
                zi += 1
                if zi == bnd1:
                    emit_batch1()
                if zi == bnd2:
                    emit_batch2()
                if zi == QI:
                    emit_q_block()
                if zi <= QI:
                    continue
            elif zi < NZ:
                raise RuntimeError("ztile_plan shorter than NZ")
            # advance phase B (front zi-stagger keeps DVE queue un-blocked)
            budget = KNOBS["b_catch"] if zi < NZ else NZ
            stag = KNOBS["b_stagger"]
            while budget > 0 and fj < NZ and (fj <= zi - 2 or zi >= NZ):
                emit_front(fj)
                if fj >= stag:
                    emit_back(fj - stag)
                fj += 1
                budget -= 1
            if zi >= NZ and fj >= NZ:
                break
        for r in range(max(NZ - KNOBS["b_stagger"], 0), NZ):
            emit_back(r)

        rcp = pacc.tile([SLOTS, 1], F32)
        nc.vector.reciprocal(rcp, den)
        out_sb = pacc.tile([SLOTS, 512], F32)
        nc.vector.tensor_scalar_mul(out_sb, numer, rcp)
        nc.sync.dma_start(y, out_sb)


_CACHE = {}


def _get_program(plan_key=None):
    if plan_key is None:
        return _CACHE["nc"], _CACHE["aps"]
    if _CACHE.get("key") == plan_key:
        return _CACHE["nc"], _CACHE["aps"]
    groups = list(plan_key)
    nc = bacc.Bacc(None, target_bir_lowering=False, debug=False)
    aps = {
        "xh": nc.dram_tensor("xh", [JC, 128, TOK_CORE], BF16, kind="ExternalInput").ap(),
        "packb": nc.dram_tensor("packb", [128, PB], BF16, kind="ExternalInput").ap(),
        "packq": nc.dram_tensor("packq", [128, 2048], BF16, kind="ExternalInput").ap(),
        "packf": nc.dram_tensor("packf", [128, PF], F32, kind="ExternalInput").ap(),
        "pack8": nc.dram_tensor("pack8", [1, 1280], FP8, kind="ExternalInput").ap(),
        "y": nc.dram_tensor("y", [SLOTS, 512], F32, kind="ExternalOutput").ap(),
    }
    with tile.TileContext(nc) as tc:
        _build_kernel_body(tc, aps, groups)
    nc.finalize()
    _CACHE["key"] = plan_key
    _CACHE["nc"] = nc
    _CACHE["aps"] = aps
    return nc, aps


def _make_in_maps(hidden_states, Wq, bq, Wk, bk, lengths, batch_of, K, groups):
    hidden = np.asarray(hidden_states, dtype=np.float32)
    Wq = np.asarray(Wq, dtype=np.float32)
    Wk = np.asarray(Wk, dtype=np.float32)
    bqv = np.asarray(bq, dtype=np.float32)
    bkv = np.asarray(bk, dtype=np.float32)
    lens = np.asarray(lengths).astype(np.int64)

    packb = np.zeros((128, PB), dtype=BF16NP)
    p = np.arange(128)
    packb[:, OB_WK : OB_WK + 2048] = (
        np.ascontiguousarray(Wk.T).reshape(JC, 128, H).transpose(1, 0, 2).reshape(128, 2048)
    ).astype(BF16NP)
    packq = (
        (np.ascontiguousarray(Wq.T) / S)
        .reshape(JC, 128, H)
        .transpose(1, 0, 2)
        .reshape(128, 2048)
    ).astype(BF16NP)
    sel = np.zeros((128, 512), dtype=BF16NP)
    for g in range(SLOTS):
        sel[g, g * 128 : (g + 1) * 128] = BF16NP(1.0)
    packb[:, OB_SEL : OB_SEL + 512] = sel
    for g in range(SLOTS):
        packb[:, OB_IND + g * SLOTS + g] = BF16NP(1.0)
    packb[0, OB_ONESR : OB_ONESR + 128] = BF16NP(1.0)
    packb[:, OB_ONESC] = BF16NP(1.0)
    packb[0, OB_BK : OB_BK + 512] = bkv.astype(BF16NP)

    base_packf = np.zeros((128, PF), dtype=np.float32)
    base_packf[0:SLOTS, OF_BQ : OF_BQ + 512] = bqv[None, :]

    pack8 = np.zeros((1, 1280), dtype=FP8NP)
    pack8[0, 0:256] = FP8NP(1.0)
    # two-term fp8 split: k-tile 0 = fp8(bk), k-tile 1 = fp8(bk - fp8(bk)),
    # so the DoubleRow bias matmul reproduces bk to ~1e-4.
    bk8 = bkv.astype(FP8NP)
    pack8[0, 256 : 256 + 512] = bk8
    pack8[0, 768 : 768 + 512] = (bkv - bk8.astype(np.float32)).astype(FP8NP)

    in_maps = []
    for c in range(NCORES):
        hs = hidden[:, batch_of[c], :]  # [S, 4, H]
        xh = (
            hs.transpose(2, 1, 0).reshape(JC, 128, SLOTS, S).reshape(JC, 128, TOK_CORE)
        ).astype(BF16NP)
        packf = base_packf.copy()
        zi = 0
        for s, hh, nz in groups:
            ln = int(lens[batch_of[c, s]])
            for t in range(nz):
                s0 = hh * CHUNK + t * 128
                valid = (s0 + p) < ln
                packf[:, OF_MASK + zi] = np.where(valid, 0.0, MASK_NEG)
                zi += 1
        in_maps.append(
            {
                "xh": np.ascontiguousarray(xh),
                "packb": packb,
                "packq": packq,
                "packf": packf,
                "pack8": pack8,
            }
        )
    return in_maps


def run(hidden_states, Wq, bq, Wk, bk, lengths, trace=False):
    batch_of, K, groups = _plan(lengths)
    nc, _ = _get_program(tuple(groups))
    in_maps = _make_in_maps(
        hidden_states, Wq, bq, Wk, bk, lengths, batch_of, K, groups
    )
    res = run_bass_kernel_spmd(nc, in_maps, core_ids=list(range(NCORES)), trace=trace)
    out = np.zeros((B, H), dtype=np.float32)
    for c in range(NCORES):
        yc = np.asarray(res.results[c]["y"], dtype=np.float32)
        for s in range(SLOTS):
            out[batch_of[c, s]] = yc[s]
    return out, res


def kernel(hidden_states, Wq, bq, Wk, bk, lengths):
    out, _ = run(hidden_states, Wq, bq, Wk, bk, lengths)
    return out


# revision 22
# speedup vs baseline: 1.0157x; 1.0157x over previous
"""Trainium2 Bass kernel for nn_Attention_82660940579436 (v2).

Computation (see reference):
    q     = mean_s(hidden @ Wq.T + bq)            [B, H]
    key   = tanh(hidden @ Wk.T + bk)              [S, B, H]
    score = einsum('bsh,bh->bs', key, q) + mask   [B, S]
    out   = softmax(score) @ key                  [B, H]

Key observations driving this version:
  * Tokens with s >= lengths[b] get softmax weight exactly 0, so keys /
    scores / weighted sums are only needed for s < lengths[b] (a PREFIX of
    each batch's tokens).  Only the q-mean needs every token.
  * The host can pre-transpose + pre-cast hidden to bf16 "hT" layout
    [jc, j, tok] so the device does ZERO transposes: the z matmul consumes
    hT chunks as the stationary operand directly from DMA.
  * Batches are assigned to (core, slot) so that each slot's max length
    (over cores) is small: sort lengths desc, slot s takes ranks [8s, 8s+8).
    All cores then run the SAME program shape (z-tile counts per slot are
    global maxima); per-core data (hT, masks) differs.

Device program per core (4 slots x 4096 tokens; z-tiles of 128 tokens):
  Phase A, per 2048-token chunk-group (8 groups, z-rich first):
    - 4 HWDGE DMAs load hT chunks [128 j, 2048 tok] bf16 (one per j-chunk)
    - mean: per chunk, fold-tree (DVE) or Copy+accum (ACT) -> csum [128,1];
      m[j, (jc,g)] = csum_h0 + csum_h1 (bf16)
    - per z-tile: PE bias matmul (ones x bk) + 4 z matmuls (hT chunk
      stationary, WkT moving) -> PSUM [128 tok, 512]; ACT tanh -> keys bf16
  q block (emitted mid z-stream so PE reaches it right as the mean lands):
    q = m @ (WqT/S) (PE, bf16) ; q += bq (DVE, reads PSUM) ;
    qrep_g = sel_g.T @ q (PE) -> SBUF bf16 [128, 512] per slot
  Phase B, per z-tile:
    prod = keys[t] * qrep_slot   (DVE 2x / Pool split)
    score = rowsum(prod)         (DVE fold-tree / ACT accum split)
    e = exp(score + mask)        (ACT; mask -60 for invalid tokens)
    ei = ind_slot * e            (DVE tensor_scalar [128,4] bf16)
    numer += ei.T @ keys[t] ; den += ei.T @ ones   (PE, PSUM accumulate)
  out = numer / den -> DMA

Cost-model notes (TimelineSim/InstructionCostModel is the graded metric):
  matmul = out_free x 0.4167ns (bf16, warm); DMA = desc/16 x elem/22.5 (2x
  penalty below 512B runs -- hence 2048-token bf16 chunk rows); DVE
  TensorTensor bf16 SBUF = 2x mode; TensorReduce = 1x; ACT = 1/cycle
  + ~185ns init, accum_out +187ns.  fp8 DoubleRow would halve PE but
  measures 3.9e-2 rel err (> 2e-2 gate) -- rejected.
"""

import sys

import numpy as np

if "/opt/trn_rl_repo" not in sys.path:
    sys.path.append("/opt/trn_rl_repo")

import ml_dtypes  # noqa: E402

FP8NP = ml_dtypes.float8_e4m3fn

import concourse.bacc as bacc  # noqa: E402
import concourse.mybir as mybir  # noqa: E402
import concourse.tile as tile  # noqa: E402
from concourse.bass_utils import run_bass_kernel_spmd  # noqa: E402

S, B, H = 4096, 32, 512
NCORES = 8
SLOTS = 4  # batches per core
SLOT_TOK = S  # tokens per slot
CHUNK = 2048  # tokens per DMA chunk (4KB bf16 rows: no <512B DMA penalty)
JC = H // 128  # 4 j-chunks
TOK_CORE = SLOTS * SLOT_TOK

F32 = mybir.dt.float32
BF16 = mybir.dt.bfloat16
FP8 = mybir.dt.float8e4
AF = mybir.ActivationFunctionType
ALU = mybir.AluOpType
BF16NP = ml_dtypes.bfloat16
MASK_NEG = -60.0

# bf16 const pack offsets (elements); WqT/S ships separately (packq) so the
# startup-critical const DMA stays small.
OB_WK = 0  # [128, 4*512] WkT chunks
OB_SEL = 2048  # [4, 4*128] qrep selectors
OB_IND = 2560  # [128, 4*4] slot indicators
OB_ONESR = 2576  # [1, 128] ones row
OB_ONESC = 2704  # [128, 1] ones col
OB_BK = 2705  # [1, 512] bk
PB = 3217
# f32 const pack offsets
PF_PAD = 80  # mask columns (>= NZ)
OF_MASK = 0  # [128, PF_PAD]
OF_ZERO = PF_PAD  # [128, 1]
OF_BQ = PF_PAD + 1  # [4, 512] bq rows
PF = PF_PAD + 1 + 512

KNOBS = {
    "zps_bufs": 2,
    "fullz": 4,  # z-groups loaded full via the 2-buf xf pool (serve mean too)
    "q_after": 35,  # emit q block after this many z-tiles (min: first 2 groups)
    "b_catch": 4,  # phase-B tiles advanced per z-tile once past QI
    "b_stagger": 3,  # numer/TSP trail the mul/fold front by this many tiles
    "mul_pool_mod": 2,  # z-tile zi uses Pool mul when zi % mod == mod-1
    "red_act_mod": 3,  # z-tile zi reduces via ACT accum when zi % mod == 1
}

import json as _json
import os as _os

if _os.environ.get("KERNEL_KNOBS"):
    KNOBS.update(_json.loads(_os.environ["KERNEL_KNOBS"]))


def _plan(lengths):
    lens = np.asarray(lengths).astype(np.int64)
    order = np.argsort(-lens, kind="stable")
    batch_of = np.zeros((NCORES, SLOTS), dtype=np.int64)
    for s in range(SLOTS):
        for c in range(NCORES):
            batch_of[c, s] = order[NCORES * s + c]
    K = []
    for s in range(SLOTS):
        mx = int(lens[order[NCORES * s : NCORES * (s + 1)]].max())
        K.append(min(32, -(-mx // 128)))
    groups = []  # (slot, half, nz)
    for s in range(SLOTS):
        for hh in range(2):
            nz = max(0, min(16, K[s] - 16 * hh))
            groups.append((s, hh, nz))
    groups.sort(key=lambda x: (-x[2], x[0], x[1]))
    return batch_of, K, groups


def _build_kernel_body(tc, aps, groups):
    nc = tc.nc
    xh, packb, packf, y = aps["xh"], aps["packb"], aps["packf"], aps["y"]
    NZ = sum(g[2] for g in groups)

    zgroups = [g for g in groups if g[2] > 0]  # z-order (nz desc)
    mgroups = [g for g in groups if g[2] == 0]  # mean-only
    NFULL = min(KNOBS["fullz"], len(zgroups))
    fullz = zgroups[:NFULL]
    trimz = zgroups[NFULL:]
    dls = mgroups + trimz  # groups whose full chunk loads via the dl pool

    from contextlib import ExitStack

    with ExitStack() as ctx:
        consts = ctx.enter_context(tc.tile_pool(name="consts", bufs=1))
        pxf = ctx.enter_context(tc.tile_pool(name="xf", bufs=2))
        pdl = ctx.enter_context(tc.tile_pool(name="dl", bufs=2))
        ptz = ctx.enter_context(tc.tile_pool(name="tz", bufs=1))
        pkeys = ctx.enter_context(tc.tile_pool(name="keys", bufs=max(NZ, 1)))
        pfold = ctx.enter_context(tc.tile_pool(name="fold", bufs=4))
        pascr = ctx.enter_context(tc.tile_pool(name="ascr", bufs=2))
        pprod = ctx.enter_context(tc.tile_pool(name="prod", bufs=4))
        psmall = ctx.enter_context(tc.tile_pool(name="small", bufs=6))
        pacc = ctx.enter_context(tc.tile_pool(name="acc", bufs=1))
        ps_z = ctx.enter_context(
            tc.tile_pool(name="ps_z", bufs=KNOBS["zps_bufs"], space="PSUM")
        )
        ps_q = ctx.enter_context(tc.tile_pool(name="ps_q", bufs=1, space="PSUM"))
        ps_qr = ctx.enter_context(tc.tile_pool(name="ps_qr", bufs=2, space="PSUM"))
        ps_acc = ctx.enter_context(tc.tile_pool(name="ps_acc", bufs=1, space="PSUM"))

        cb = consts.tile([128, PB], BF16)
        # small consts (bk/ones/ind/sel) land in ~1us; WK chunks follow
        # interleaved with group 0's loads so the PE starts at ~3us.
        nc.sync.dma_start(cb[:, 2048:PB], packb[:, 2048:PB])
        cf = consts.tile([128, PF], F32)
        cq = consts.tile([128, 2048], BF16)  # WqT/S; DMA deferred
        c8 = consts.tile([1, 1280], FP8)
        nc.sync.dma_start(c8, aps["pack8"])
        ones8_dr = c8[0:1, 0:256].rearrange("p (two f) -> p two f", two=2)
        bk8_dr = c8[0:1, 256:1280].rearrange("p (two f) -> p two f", two=2)

        def wk_sb(c):
            return cb[:, OB_WK + c * 512 : OB_WK + (c + 1) * 512]

        def wq_sb(c):
            return cq[:, c * 512 : (c + 1) * 512]

        def sel_sb(g):
            return cb[0:SLOTS, OB_SEL + g * 128 : OB_SEL + (g + 1) * 128]

        def ind_sb(g):
            return cb[:, OB_IND + g * SLOTS : OB_IND + (g + 1) * SLOTS]

        ones_row = cb[0:1, OB_ONESR : OB_ONESR + 128]
        ones_col = cb[:, OB_ONESC : OB_ONESC + 1]
        bk_row = cb[0:1, OB_BK : OB_BK + 512]
        mask_sb = cf[:, OF_MASK : OF_MASK + PF_PAD]
        zero_sb = cf[:, OF_ZERO : OF_ZERO + 1]
        bq_sb = cf[0:SLOTS, OF_BQ : OF_BQ + 512]

        m_sb = pacc.tile([128, SLOTS * JC], BF16)  # col = jc*4 + g
        mparts = pacc.tile([128, 2 * SLOTS * JC], F32)  # col = (jc*4+g)*2 + half

        # ---------------- emission helpers ----------------
        def emit_mean(xt, s, hh):
            """chunk tiles -> csum [128,1] per jc, into mparts."""
            for jc in range(JC):
                dst = mparts[
                    :, (jc * SLOTS + s) * 2 + hh : (jc * SLOTS + s) * 2 + hh + 1
                ]
                f = pfold.tile([128, 1024], BF16, tag="fold")
                nc.vector.tensor_add(f, xt[jc][:, 0:1024], xt[jc][:, 1024:2048])
                nc.vector.tensor_add(f[:, 0:512], f[:, 0:512], f[:, 512:1024])
                nc.vector.tensor_add(f[:, 0:256], f[:, 0:256], f[:, 256:512])
                nc.vector.tensor_add(f[:, 0:128], f[:, 0:128], f[:, 128:256])
                nc.vector.tensor_reduce(
                    dst, f[:, 0:128], axis=mybir.AxisListType.X, op=ALU.add
                )

        def load_full(s, hh):
            base = s * SLOT_TOK + hh * CHUNK
            xt = []
            for jc in range(JC):
                t = pxf.tile([128, CHUNK], BF16, tag=f"xf{jc}")
                nc.sync.dma_start(t, xh[jc, :, base : base + CHUNK])
                xt.append(t)
            emit_mean(xt, s, hh)
            return xt

        def load_dl(s, hh):
            base = s * SLOT_TOK + hh * CHUNK
            xt = []
            for jc in range(JC):
                t = pdl.tile([128, CHUNK], BF16, tag=f"dl{jc}")
                nc.sync.dma_start(t, xh[jc, :, base : base + CHUNK])
                xt.append(t)
            emit_mean(xt, s, hh)

        def load_trim(idx, s, hh, nz):
            base = s * SLOT_TOK + hh * CHUNK
            w = nz * 128
            xt = []
            for jc in range(JC):
                t = ptz.tile([128, w], BF16, tag=f"tz{idx}_{jc}")
                nc.sync.dma_start(t, xh[jc, :, base : base + w])
                xt.append(t)
            return xt

        def emit_madds():
            for col in range(SLOTS * JC):
                nc.vector.tensor_add(
                    m_sb[:, col : col + 1],
                    mparts[:, 2 * col : 2 * col + 1],
                    mparts[:, 2 * col + 1 : 2 * col + 2],
                )

        qreps = []

        def emit_q_block():
            q_ps = ps_q.tile([SLOTS, 512], F32, tag="q")
            for jc in range(JC):
                nc.tensor.matmul(
                    q_ps,
                    m_sb[:, jc * SLOTS : (jc + 1) * SLOTS],
                    wq_sb(jc),
                    start=(jc == 0),
                    stop=(jc == JC - 1),
                )
            q_sbt = pacc.tile([SLOTS, 512], BF16)
            nc.vector.tensor_add(q_sbt, q_ps, bq_sb)
            for g in range(SLOTS):
                qr_ps = ps_qr.tile([128, 512], F32, tag="qr")
                nc.tensor.matmul(qr_ps, sel_sb(g), q_sbt, start=True, stop=True)
                qr = pacc.tile([128, 512], BF16, tag=f"qrep{g}")
                nc.vector.tensor_copy(qr, qr_ps)
                qreps.append(qr)

        # ---------------- phase B emitters (front/back stagger) ----------------
        numer = ps_acc.tile([SLOTS, 512], F32, tag="numer")
        den = ps_acc.tile([SLOTS, 1], F32, tag="den")
        keys = []
        zslot = []
        e_tiles = []

        def emit_front(zi):
            kt = keys[zi]
            s = zslot[zi]
            prod = pprod.tile([128, 512], BF16, tag="prod")
            if zi % KNOBS["mul_pool_mod"] == KNOBS["mul_pool_mod"] - 1:
                nc.gpsimd.tensor_mul(prod, kt, qreps[s])
            else:
                nc.vector.tensor_mul(prod, kt, qreps[s])
            sc = psmall.tile([128, 1], F32, tag="sc")
            ram = KNOBS["red_act_mod"]
            if ram and zi % ram == 1:
                scr = pascr.tile([128, 512], BF16, tag="bscr")
                nc.scalar.activation(scr, prod, AF.Copy, accum_out=sc)
            else:
                nc.vector.tensor_add(prod[:, 0:256], prod[:, 0:256], prod[:, 256:512])
                nc.vector.tensor_add(prod[:, 0:128], prod[:, 0:128], prod[:, 128:256])
                nc.vector.tensor_reduce(
                    sc, prod[:, 0:128], axis=mybir.AxisListType.X, op=ALU.add
                )
            e_t = psmall.tile([128, 1], F32, tag="e")
            nc.scalar.activation(e_t, sc, AF.Exp, bias=mask_sb[:, zi : zi + 1])
            e_tiles.append(e_t)

        def emit_back(zi):
            kt = keys[zi]
            ei = psmall.tile([128, SLOTS], BF16, tag="ei")
            nc.vector.tensor_scalar_mul(ei, ind_sb(zslot[zi]), e_tiles[zi])
            nc.tensor.matmul(numer, ei, kt, start=(zi == 0), stop=(zi == NZ - 1))
            nc.tensor.matmul(den, ei, ones_col, start=(zi == 0), stop=(zi == NZ - 1))

        # ---------------- the merged A/B schedule ----------------
        # DMA issue order (SP queue is FIFO): full z-groups interleaved with
        # dl (mean-copy) loads so every mean source has landed by ~40us while
        # the PE never waits for its next z chunk.
        ztile_plan = []  # (xt, local t, slot)

        def plan_group(xt, s, nz):
            for t in range(nz):
                ztile_plan.append((xt, t, s))

        # batch0 inline: WK slices first, then group 0
        for jc in range(JC):
            nc.sync.dma_start(
                cb[:, OB_WK + jc * 512 : OB_WK + (jc + 1) * 512],
                packb[:, OB_WK + jc * 512 : OB_WK + (jc + 1) * 512],
            )
        nst = 0
        xt0 = load_full(*fullz[0][:2])
        nc.sync.dma_start(cf, packf)
        plan_group(xt0, fullz[0][0], fullz[0][2])
        if NFULL > 1:
            xt1 = load_full(*fullz[1][:2])
            plan_group(xt1, fullz[1][0], fullz[1][2])
        for d in dls[0:2]:
            load_dl(d[0], d[1])
        # batch1/2 described as thunks, emitted at group boundaries
        def emit_batch1():
            if NFULL > 2:
                xt = load_full(*fullz[2][:2])
                plan_group(xt, fullz[2][0], fullz[2][2])
            for d in dls[2:4]:
                load_dl(d[0], d[1])

        def emit_batch2():
            if NFULL > 3:
                xt = load_full(*fullz[3][:2])
                plan_group(xt, fullz[3][0], fullz[3][2])
            nc.sync.dma_start(cq, aps["packq"])
            for d in dls[4:]:
                load_dl(d[0], d[1])
            for i, (s, hh, nz) in enumerate(trimz):
                xt = load_trim(i, s, hh, nz)
                plan_group(xt, s, nz)
            emit_madds()

        tiles01 = nst + fullz[0][2] + (fullz[1][2] if NFULL > 1 else 0)
        QI = max(min(KNOBS["q_after"], NZ - 1), min(tiles01 + 1, NZ - 1))
        bnd1 = nst + fullz[0][2]  # after group 0's tiles
        bnd2 = tiles01  # after group 1's tiles

        zi = 0
        fj = 0  # phase B front progress

        def emit_ztile(xt, t, s):
            zp = ps_z.tile([128, 512], F32, tag="z")
            # fp8 DoubleRow rank-2 bias: out = sum_i ones8[:,i,:].T @ bk8[:,i,:]
            # = bk broadcast over tokens, at 0.5 cycles/row (half the bf16 cost)
            nc.tensor.matmul(
                zp,
                ones8_dr,
                bk8_dr,
                start=True,
                stop=False,
                perf_mode=mybir.MatmulPerfMode.DoubleRow,
            )
            for jc in range(JC):
                nc.tensor.matmul(
                    zp,
                    xt[jc][:, t * 128 : (t + 1) * 128],
                    wk_sb(jc),
                    start=False,
                    stop=(jc == JC - 1),
                )
            kt = pkeys.tile([128, 512], BF16, tag="key")
            nc.scalar.activation(kt, zp, AF.Tanh, bias=zero_sb)
            keys.append(kt)
            zslot.append(s)

        while zi < NZ or fj < NZ:
            if zi < len(ztile_plan):
                emit_ztile(*ztile_plan[zi])
                zi += 1
                if zi == bnd1:
                    emit_batch1()
                if zi == bnd2:
                    emit_batch2()
                if zi == QI:
                    emit_q_block()
                if zi <= QI:
                    continue
            elif zi < NZ:
                raise RuntimeError("ztile_plan shorter than NZ")
            # advance phase B (front zi-stagger keeps DVE queue un-blocked)
            budget = KNOBS["b_catch"] if zi < NZ else NZ
            stag = KNOBS["b_stagger"]
            while budget > 0 and fj < NZ and (fj <= zi - 2 or zi >= NZ):
                emit_front(fj)
                if fj >= stag:
                    emit_back(fj - stag)
                fj += 1
                budget -= 1
            if zi >= NZ and fj >= NZ:
                break
        for r in range(max(NZ - KNOBS["b_stagger"], 0), NZ):
            emit_back(r)

        rcp = pacc.tile([SLOTS, 1], F32)
        nc.vector.reciprocal(rcp, den)
        out_sb = pacc.tile([SLOTS, 512], F32)
        nc.vector.tensor_scalar_mul(out_sb, numer, rcp)
        nc.sync.dma_start(y, out_sb)


_CACHE = {}


def _get_program(plan_key=None):
    if plan_key is None:
        return _CACHE["nc"], _CACHE["aps"]
    if _CACHE.get("key") == plan_key:
        return _CACHE["nc"], _CACHE["aps"]
    groups = list(plan_key)
    nc = bacc.Bacc(None, target_bir_lowering=False, debug=False)
    aps = {
        "xh": nc.dram_tensor("xh", [JC, 128, TOK_CORE], BF16, kind="ExternalInput").ap(),
        "packb": nc.dram_tensor("packb", [128, PB], BF16, kind="ExternalInput").ap(),
        "packq": nc.dram_tensor("packq", [128, 2048], BF16, kind="ExternalInput").ap(),
        "packf": nc.dram_tensor("packf", [128, PF], F32, kind="ExternalInput").ap(),
        "pack8": nc.dram_tensor("pack8", [1, 1280], FP8, kind="ExternalInput").ap(),
        "y": nc.dram_tensor("y", [SLOTS, 512], F32, kind="ExternalOutput").ap(),
    }
    with tile.TileContext(nc) as tc:
        _build_kernel_body(tc, aps, groups)
    nc.finalize()
    _CACHE["key"] = plan_key
    _CACHE["nc"] = nc
    _CACHE["aps"] = aps
    return nc, aps


def _make_in_maps(hidden_states, Wq, bq, Wk, bk, lengths, batch_of, K, groups):
    hidden = np.asarray(hidden_states, dtype=np.float32)
    Wq = np.asarray(Wq, dtype=np.float32)
    Wk = np.asarray(Wk, dtype=np.float32)
    bqv = np.asarray(bq, dtype=np.float32)
    bkv = np.asarray(bk, dtype=np.float32)
    lens = np.asarray(lengths).astype(np.int64)

    packb = np.zeros((128, PB), dtype=BF16NP)
    p = np.arange(128)
    packb[:, OB_WK : OB_WK + 2048] = (
        np.ascontiguousarray(Wk.T).reshape(JC, 128, H).transpose(1, 0, 2).reshape(128, 2048)
    ).astype(BF16NP)
    packq = (
        (np.ascontiguousarray(Wq.T) / S)
        .reshape(JC, 128, H)
        .transpose(1, 0, 2)
        .reshape(128, 2048)
    ).astype(BF16NP)
    sel = np.zeros((128, 512), dtype=BF16NP)
    for g in range(SLOTS):
        sel[g, g * 128 : (g + 1) * 128] = BF16NP(1.0)
    packb[:, OB_SEL : OB_SEL + 512] = sel
    for g in range(SLOTS):
        packb[:, OB_IND + g * SLOTS + g] = BF16NP(1.0)
    packb[0, OB_ONESR : OB_ONESR + 128] = BF16NP(1.0)
    packb[:, OB_ONESC] = BF16NP(1.0)
    packb[0, OB_BK : OB_BK + 512] = bkv.astype(BF16NP)

    base_packf = np.zeros((128, PF), dtype=np.float32)
    base_packf[0:SLOTS, OF_BQ : OF_BQ + 512] = bqv[None, :]

    pack8 = np.zeros((1, 1280), dtype=FP8NP)
    # two-term fp8 split with a scaled residual: k-tile 0 = 1 * fp8(bk),
    # k-tile 1 = (1/16) * fp8(16*(bk - fp8(bk))), reproducing bk to ~1e-5.
    pack8[0, 0:128] = FP8NP(1.0)
    pack8[0, 128:256] = FP8NP(1.0 / 16.0)
    bk8 = bkv.astype(FP8NP)
    pack8[0, 256 : 256 + 512] = bk8
    pack8[0, 768 : 768 + 512] = (
        16.0 * (bkv - bk8.astype(np.float32))
    ).astype(FP8NP)

    in_maps = []
    for c in range(NCORES):
        hs = hidden[:, batch_of[c], :]  # [S, 4, H]
        xh = (
            hs.transpose(2, 1, 0).reshape(JC, 128, SLOTS, S).reshape(JC, 128, TOK_CORE)
        ).astype(BF16NP)
        packf = base_packf.copy()
        zi = 0
        for s, hh, nz in groups:
            ln = int(lens[batch_of[c, s]])
            for t in range(nz):
                s0 = hh * CHUNK + t * 128
                valid = (s0 + p) < ln
                packf[:, OF_MASK + zi] = np.where(valid, 0.0, MASK_NEG)
                zi += 1
        in_maps.append(
            {
                "xh": np.ascontiguousarray(xh),
                "packb": packb,
                "packq": packq,
                "packf": packf,
                "pack8": pack8,
            }
        )
    return in_maps


def run(hidden_states, Wq, bq, Wk, bk, lengths, trace=False):
    batch_of, K, groups = _plan(lengths)
    nc, _ = _get_program(tuple(groups))
    in_maps = _make_in_maps(
        hidden_states, Wq, bq, Wk, bk, lengths, batch_of, K, groups
    )
    res = run_bass_kernel_spmd(nc, in_maps, core_ids=list(range(NCORES)), trace=trace)
    out = np.zeros((B, H), dtype=np.float32)
    for c in range(NCORES):
        yc = np.asarray(res.results[c]["y"], dtype=np.float32)
        for s in range(SLOTS):
            out[batch_of[c, s]] = yc[s]
    return out, res


def kernel(hidden_states, Wq, bq, Wk, bk, lengths):
    out, _ = run(hidden_states, Wq, bq, Wk, bk, lengths)
    return out


# revision 23
# speedup vs baseline: 1.0233x; 1.0076x over previous
"""Trainium2 Bass kernel for nn_Attention_82660940579436 (v2).

Computation (see reference):
    q     = mean_s(hidden @ Wq.T + bq)            [B, H]
    key   = tanh(hidden @ Wk.T + bk)              [S, B, H]
    score = einsum('bsh,bh->bs', key, q) + mask   [B, S]
    out   = softmax(score) @ key                  [B, H]

Key observations driving this version:
  * Tokens with s >= lengths[b] get softmax weight exactly 0, so keys /
    scores / weighted sums are only needed for s < lengths[b] (a PREFIX of
    each batch's tokens).  Only the q-mean needs every token.
  * The host can pre-transpose + pre-cast hidden to bf16 "hT" layout
    [jc, j, tok] so the device does ZERO transposes: the z matmul consumes
    hT chunks as the stationary operand directly from DMA.
  * Batches are assigned to (core, slot) so that each slot's max length
    (over cores) is small: sort lengths desc, slot s takes ranks [8s, 8s+8).
    All cores then run the SAME program shape (z-tile counts per slot are
    global maxima); per-core data (hT, masks) differs.

Device program per core (4 slots x 4096 tokens; z-tiles of 128 tokens):
  Phase A, per 2048-token chunk-group (8 groups, z-rich first):
    - 4 HWDGE DMAs load hT chunks [128 j, 2048 tok] bf16 (one per j-chunk)
    - mean: per chunk, fold-tree (DVE) or Copy+accum (ACT) -> csum [128,1];
      m[j, (jc,g)] = csum_h0 + csum_h1 (bf16)
    - per z-tile: PE bias matmul (ones x bk) + 4 z matmuls (hT chunk
      stationary, WkT moving) -> PSUM [128 tok, 512]; ACT tanh -> keys bf16
  q block (emitted mid z-stream so PE reaches it right as the mean lands):
    q = m @ (WqT/S) (PE, bf16) ; q += bq (DVE, reads PSUM) ;
    qrep_g = sel_g.T @ q (PE) -> SBUF bf16 [128, 512] per slot
  Phase B, per z-tile:
    prod = keys[t] * qrep_slot   (DVE 2x / Pool split)
    score = rowsum(prod)         (DVE fold-tree / ACT accum split)
    e = exp(score + mask)        (ACT; mask -60 for invalid tokens)
    ei = ind_slot * e            (DVE tensor_scalar [128,4] bf16)
    numer += ei.T @ keys[t] ; den += ei.T @ ones   (PE, PSUM accumulate)
  out = numer / den -> DMA

Cost-model notes (TimelineSim/InstructionCostModel is the graded metric):
  matmul = out_free x 0.4167ns (bf16, warm); DMA = desc/16 x elem/22.5 (2x
  penalty below 512B runs -- hence 2048-token bf16 chunk rows); DVE
  TensorTensor bf16 SBUF = 2x mode; TensorReduce = 1x; ACT = 1/cycle
  + ~185ns init, accum_out +187ns.  fp8 DoubleRow would halve PE but
  measures 3.9e-2 rel err (> 2e-2 gate) -- rejected.
"""

import sys

import numpy as np

if "/opt/trn_rl_repo" not in sys.path:
    sys.path.append("/opt/trn_rl_repo")

import ml_dtypes  # noqa: E402

FP8NP = ml_dtypes.float8_e4m3fn

import concourse.bacc as bacc  # noqa: E402
import concourse.mybir as mybir  # noqa: E402
import concourse.tile as tile  # noqa: E402
from concourse.bass_utils import run_bass_kernel_spmd  # noqa: E402

S, B, H = 4096, 32, 512
NCORES = 8
SLOTS = 4  # batches per core
SLOT_TOK = S  # tokens per slot
CHUNK = 2048  # tokens per DMA chunk (4KB bf16 rows: no <512B DMA penalty)
JC = H // 128  # 4 j-chunks
TOK_CORE = SLOTS * SLOT_TOK

F32 = mybir.dt.float32
BF16 = mybir.dt.bfloat16
FP8 = mybir.dt.float8e4
AF = mybir.ActivationFunctionType
ALU = mybir.AluOpType
BF16NP = ml_dtypes.bfloat16
MASK_NEG = -60.0

# bf16 const pack offsets (elements); WqT/S ships separately (packq) so the
# startup-critical const DMA stays small.
OB_WK = 0  # [128, 4*512] WkT chunks
OB_SEL = 2048  # [4, 4*128] qrep selectors
OB_IND = 2560  # [128, 4*4] slot indicators
OB_ONESR = 2576  # [1, 128] ones row
OB_ONESC = 2704  # [128, 1] ones col
OB_BK = 2705  # [1, 512] bk
PB = 3217
# f32 const pack offsets
PF_PAD = 80  # mask columns (>= NZ)
OF_MASK = 0  # [128, PF_PAD]
OF_ZERO = PF_PAD  # [128, 1]
OF_BQ = PF_PAD + 1  # [4, 512] bq rows
PF = PF_PAD + 1 + 512

KNOBS = {
    "zps_bufs": 2,
    "fullz": 4,  # z-groups loaded full via the 2-buf xf pool (serve mean too)
    "q_after": 35,  # emit q block after this many z-tiles (min: first 2 groups)
    "b_catch": 4,  # phase-B tiles advanced per z-tile once past QI
    "b_stagger": 3,  # numer/TSP trail the mul/fold front by this many tiles
    "mul_pool_mod": 2,  # z-tile zi uses Pool mul when zi % mod == mod-1
    "red_act_mod": 3,  # z-tile zi reduces via ACT accum when zi % mod == 1
}

import json as _json
import os as _os

if _os.environ.get("KERNEL_KNOBS"):
    KNOBS.update(_json.loads(_os.environ["KERNEL_KNOBS"]))


def _plan(lengths):
    lens = np.asarray(lengths).astype(np.int64)
    order = np.argsort(-lens, kind="stable")
    batch_of = np.zeros((NCORES, SLOTS), dtype=np.int64)
    for s in range(SLOTS):
        for c in range(NCORES):
            batch_of[c, s] = order[NCORES * s + c]
    K = []
    for s in range(SLOTS):
        mx = int(lens[order[NCORES * s : NCORES * (s + 1)]].max())
        K.append(min(32, -(-mx // 128)))
    groups = []  # (slot, half, nz)
    for s in range(SLOTS):
        for hh in range(2):
            nz = max(0, min(16, K[s] - 16 * hh))
            groups.append((s, hh, nz))
    groups.sort(key=lambda x: (-x[2], x[0], x[1]))
    return batch_of, K, groups


def _build_kernel_body(tc, aps, groups):
    nc = tc.nc
    xh, packb, packf, y = aps["xh"], aps["packb"], aps["packf"], aps["y"]
    NZ = sum(g[2] for g in groups)

    zgroups = [g for g in groups if g[2] > 0]  # z-order (nz desc)
    mgroups = [g for g in groups if g[2] == 0]  # mean-only
    NFULL = min(KNOBS["fullz"], len(zgroups))
    fullz = zgroups[:NFULL]
    trimz = zgroups[NFULL:]
    dls = mgroups + trimz  # groups whose full chunk loads via the dl pool

    from contextlib import ExitStack

    with ExitStack() as ctx:
        consts = ctx.enter_context(tc.tile_pool(name="consts", bufs=1))
        pxf = ctx.enter_context(tc.tile_pool(name="xf", bufs=2))
        pdl = ctx.enter_context(tc.tile_pool(name="dl", bufs=2))
        ptz = ctx.enter_context(tc.tile_pool(name="tz", bufs=1))
        pkeys = ctx.enter_context(tc.tile_pool(name="keys", bufs=max(NZ, 1)))
        pfold = ctx.enter_context(tc.tile_pool(name="fold", bufs=4))
        pascr = ctx.enter_context(tc.tile_pool(name="ascr", bufs=2))
        pprod = ctx.enter_context(tc.tile_pool(name="prod", bufs=4))
        psmall = ctx.enter_context(tc.tile_pool(name="small", bufs=6))
        pacc = ctx.enter_context(tc.tile_pool(name="acc", bufs=1))
        ps_z = ctx.enter_context(
            tc.tile_pool(name="ps_z", bufs=KNOBS["zps_bufs"], space="PSUM")
        )
        ps_q = ctx.enter_context(tc.tile_pool(name="ps_q", bufs=1, space="PSUM"))
        ps_qr = ctx.enter_context(tc.tile_pool(name="ps_qr", bufs=2, space="PSUM"))
        ps_acc = ctx.enter_context(tc.tile_pool(name="ps_acc", bufs=1, space="PSUM"))

        cb = consts.tile([128, PB], BF16)
        # small consts (bk/ones/ind/sel) land in ~1us; WK chunks follow
        # interleaved with group 0's loads so the PE starts at ~3us.
        nc.sync.dma_start(cb[:, 2048:PB], packb[:, 2048:PB])
        cf = consts.tile([128, PF], F32)
        cq = consts.tile([128, 2048], BF16)  # WqT/S; DMA deferred
        c8 = consts.tile([1, 1280], FP8)
        nc.sync.dma_start(c8, aps["pack8"])
        ones8_dr = c8[0:1, 0:256].rearrange("p (two f) -> p two f", two=2)
        bk8_dr = c8[0:1, 256:1280].rearrange("p (two f) -> p two f", two=2)

        def wk_sb(c):
            return cb[:, OB_WK + c * 512 : OB_WK + (c + 1) * 512]

        def wq_sb(c):
            return cq[:, c * 512 : (c + 1) * 512]

        def sel_sb(g):
            return cb[0:SLOTS, OB_SEL + g * 128 : OB_SEL + (g + 1) * 128]

        def ind_sb(g):
            return cb[:, OB_IND + g * SLOTS : OB_IND + (g + 1) * SLOTS]

        ones_row = cb[0:1, OB_ONESR : OB_ONESR + 128]
        ones_col = cb[:, OB_ONESC : OB_ONESC + 1]
        bk_row = cb[0:1, OB_BK : OB_BK + 512]
        mask_sb = cf[:, OF_MASK : OF_MASK + PF_PAD]
        zero_sb = cf[:, OF_ZERO : OF_ZERO + 1]
        bq_sb = cf[0:SLOTS, OF_BQ : OF_BQ + 512]

        m_sb = pacc.tile([128, SLOTS * JC], BF16)  # col = jc*4 + g
        mparts = pacc.tile([128, 2 * SLOTS * JC], F32)  # col = (jc*4+g)*2 + half

        # ---------------- emission helpers ----------------
        def emit_mean(xt, s, hh):
            """chunk tiles -> csum [128,1] per jc, into mparts."""
            for jc in range(JC):
                dst = mparts[
                    :, (jc * SLOTS + s) * 2 + hh : (jc * SLOTS + s) * 2 + hh + 1
                ]
                f = pfold.tile([128, 1024], BF16, tag="fold")
                nc.vector.tensor_add(f, xt[jc][:, 0:1024], xt[jc][:, 1024:2048])
                nc.vector.tensor_add(f[:, 0:512], f[:, 0:512], f[:, 512:1024])
                nc.vector.tensor_add(f[:, 0:256], f[:, 0:256], f[:, 256:512])
                nc.vector.tensor_add(f[:, 0:128], f[:, 0:128], f[:, 128:256])
                nc.vector.tensor_reduce(
                    dst, f[:, 0:128], axis=mybir.AxisListType.X, op=ALU.add
                )

        def load_full(s, hh):
            base = s * SLOT_TOK + hh * CHUNK
            xt = []
            for jc in range(JC):
                t = pxf.tile([128, CHUNK], BF16, tag=f"xf{jc}")
                nc.sync.dma_start(t, xh[jc, :, base : base + CHUNK])
                xt.append(t)
            emit_mean(xt, s, hh)
            return xt

        def load_dl(s, hh):
            base = s * SLOT_TOK + hh * CHUNK
            xt = []
            for jc in range(JC):
                t = pdl.tile([128, CHUNK], BF16, tag=f"dl{jc}")
                nc.sync.dma_start(t, xh[jc, :, base : base + CHUNK])
                xt.append(t)
            emit_mean(xt, s, hh)

        def load_trim(idx, s, hh, nz):
            base = s * SLOT_TOK + hh * CHUNK
            w = nz * 128
            xt = []
            for jc in range(JC):
                t = ptz.tile([128, w], BF16, tag=f"tz{idx}_{jc}")
                nc.sync.dma_start(t, xh[jc, :, base : base + w])
                xt.append(t)
            return xt

        def emit_madds():
            for col in range(SLOTS * JC):
                nc.vector.tensor_add(
                    m_sb[:, col : col + 1],
                    mparts[:, 2 * col : 2 * col + 1],
                    mparts[:, 2 * col + 1 : 2 * col + 2],
                )

        qreps = []

        def emit_q_block():
            q_ps = ps_q.tile([SLOTS, 512], F32, tag="q")
            for jc in range(JC):
                nc.tensor.matmul(
                    q_ps,
                    m_sb[:, jc * SLOTS : (jc + 1) * SLOTS],
                    wq_sb(jc),
                    start=(jc == 0),
                    stop=(jc == JC - 1),
                )
            q_sbt = pacc.tile([SLOTS, 512], BF16)
            nc.vector.tensor_add(q_sbt, q_ps, bq_sb)
            for g in range(SLOTS):
                qr_ps = ps_qr.tile([128, 512], F32, tag="qr")
                nc.tensor.matmul(qr_ps, sel_sb(g), q_sbt, start=True, stop=True)
                qr = pacc.tile([128, 512], BF16, tag=f"qrep{g}")
                nc.vector.tensor_copy(qr, qr_ps)
                qreps.append(qr)

        # ---------------- phase B emitters (front/back stagger) ----------------
        numer = ps_acc.tile([SLOTS, 512], F32, tag="numer")
        den = ps_acc.tile([SLOTS, 1], F32, tag="den")
        keys = []
        zslot = []
        e_tiles = []

        def emit_front(zi):
            kt = keys[zi]
            s = zslot[zi]
            prod = pprod.tile([128, 512], BF16, tag="prod")
            if zi % KNOBS["mul_pool_mod"] == KNOBS["mul_pool_mod"] - 1:
                nc.gpsimd.tensor_mul(prod, kt, qreps[s])
            else:
                nc.vector.tensor_mul(prod, kt, qreps[s])
            sc = psmall.tile([128, 1], F32, tag="sc")
            ram = KNOBS["red_act_mod"]
            if ram and zi % ram == 1:
                scr = pascr.tile([128, 512], BF16, tag="bscr")
                nc.scalar.activation(scr, prod, AF.Copy, accum_out=sc)
            else:
                nc.vector.tensor_add(prod[:, 0:256], prod[:, 0:256], prod[:, 256:512])
                nc.vector.tensor_add(prod[:, 0:128], prod[:, 0:128], prod[:, 128:256])
                nc.vector.tensor_reduce(
                    sc, prod[:, 0:128], axis=mybir.AxisListType.X, op=ALU.add
                )
            e_t = psmall.tile([128, 1], F32, tag="e")
            nc.scalar.activation(e_t, sc, AF.Exp, bias=mask_sb[:, zi : zi + 1])
            e_tiles.append(e_t)

        def emit_back(zi):
            kt = keys[zi]
            ei = psmall.tile([128, SLOTS], BF16, tag="ei")
            nc.vector.tensor_scalar_mul(ei, ind_sb(zslot[zi]), e_tiles[zi])
            nc.tensor.matmul(numer, ei, kt, start=(zi == 0), stop=(zi == NZ - 1))
            nc.tensor.matmul(den, ei, ones_col, start=(zi == 0), stop=(zi == NZ - 1))

        # ---------------- the merged A/B schedule ----------------
        # DMA issue order (SP queue is FIFO): full z-groups interleaved with
        # dl (mean-copy) loads so every mean source has landed by ~40us while
        # the PE never waits for its next z chunk.
        ztile_plan = []  # (xt, local t, slot)

        def plan_group(xt, s, nz):
            for t in range(nz):
                ztile_plan.append((xt, t, s))

        # batch0 inline: WK slices interleaved with group 0's chunks so the
        # PE's first z matmuls start as early as possible
        nst = 0
        s0, h0 = fullz[0][:2]
        base0 = s0 * SLOT_TOK + h0 * CHUNK
        xt0 = []
        for jc in range(JC):
            nc.sync.dma_start(
                cb[:, OB_WK + jc * 512 : OB_WK + (jc + 1) * 512],
                packb[:, OB_WK + jc * 512 : OB_WK + (jc + 1) * 512],
            )
            t0 = pxf.tile([128, CHUNK], BF16, tag=f"xf{jc}")
            nc.sync.dma_start(t0, xh[jc, :, base0 : base0 + CHUNK])
            xt0.append(t0)
        emit_mean(xt0, s0, h0)
        nc.sync.dma_start(cf, packf)
        plan_group(xt0, fullz[0][0], fullz[0][2])
        if NFULL > 1:
            xt1 = load_full(*fullz[1][:2])
            plan_group(xt1, fullz[1][0], fullz[1][2])
        for d in dls[0:2]:
            load_dl(d[0], d[1])
        # batch1/2 described as thunks, emitted at group boundaries
        def emit_batch1():
            if NFULL > 2:
                xt = load_full(*fullz[2][:2])
                plan_group(xt, fullz[2][0], fullz[2][2])
            for d in dls[2:4]:
                load_dl(d[0], d[1])

        def emit_batch2():
            if NFULL > 3:
                xt = load_full(*fullz[3][:2])
                plan_group(xt, fullz[3][0], fullz[3][2])
            nc.sync.dma_start(cq, aps["packq"])
            for d in dls[4:]:
                load_dl(d[0], d[1])
            for i, (s, hh, nz) in enumerate(trimz):
                xt = load_trim(i, s, hh, nz)
                plan_group(xt, s, nz)
            emit_madds()

        tiles01 = nst + fullz[0][2] + (fullz[1][2] if NFULL > 1 else 0)
        QI = max(min(KNOBS["q_after"], NZ - 1), min(tiles01 + 1, NZ - 1))
        bnd1 = nst + fullz[0][2]  # after group 0's tiles
        bnd2 = tiles01  # after group 1's tiles

        zi = 0
        fj = 0  # phase B front progress

        def emit_ztile(xt, t, s):
            zp = ps_z.tile([128, 512], F32, tag="z")
            # fp8 DoubleRow rank-2 bias: out = sum_i ones8[:,i,:].T @ bk8[:,i,:]
            # = bk broadcast over tokens, at 0.5 cycles/row (half the bf16 cost)
            nc.tensor.matmul(
                zp,
                ones8_dr,
                bk8_dr,
                start=True,
                stop=False,
                perf_mode=mybir.MatmulPerfMode.DoubleRow,
            )
            for jc in range(JC):
                nc.tensor.matmul(
                    zp,
                    xt[jc][:, t * 128 : (t + 1) * 128],
                    wk_sb(jc),
                    start=False,
                    stop=(jc == JC - 1),
                )
            kt = pkeys.tile([128, 512], BF16, tag="key")
            nc.scalar.activation(kt, zp, AF.Tanh, bias=zero_sb)
            keys.append(kt)
            zslot.append(s)

        while zi < NZ or fj < NZ:
            if zi < len(ztile_plan):
                emit_ztile(*ztile_plan[zi])
                zi += 1
                if zi == bnd1:
                    emit_batch1()
                if zi == bnd2:
                    emit_batch2()
                if zi == QI:
                    emit_q_block()
                if zi <= QI:
                    continue
            elif zi < NZ:
                raise RuntimeError("ztile_plan shorter than NZ")
            # advance phase B (front zi-stagger keeps DVE queue un-blocked)
            budget = KNOBS["b_catch"] if zi < NZ else NZ
            stag = KNOBS["b_stagger"]
            while budget > 0 and fj < NZ and (fj <= zi - 2 or zi >= NZ):
                emit_front(fj)
                if fj >= stag:
                    emit_back(fj - stag)
                fj += 1
                budget -= 1
            if zi >= NZ and fj >= NZ:
                break
        for r in range(max(NZ - KNOBS["b_stagger"], 0), NZ):
            emit_back(r)

        rcp = pacc.tile([SLOTS, 1], F32)
        nc.vector.reciprocal(rcp, den)
        out_sb = pacc.tile([SLOTS, 512], F32)
        nc.vector.tensor_scalar_mul(out_sb, numer, rcp)
        nc.sync.dma_start(y, out_sb)


_CACHE = {}


def _get_program(plan_key=None):
    if plan_key is None:
        return _CACHE["nc"], _CACHE["aps"]
    if _CACHE.get("key") == plan_key:
        return _CACHE["nc"], _CACHE["aps"]
    groups = list(plan_key)
    nc = bacc.Bacc(None, target_bir_lowering=False, debug=False)
    aps = {
        "xh": nc.dram_tensor("xh", [JC, 128, TOK_CORE], BF16, kind="ExternalInput").ap(),
        "packb": nc.dram_tensor("packb", [128, PB], BF16, kind="ExternalInput").ap(),
        "packq": nc.dram_tensor("packq", [128, 2048], BF16, kind="ExternalInput").ap(),
        "packf": nc.dram_tensor("packf", [128, PF], F32, kind="ExternalInput").ap(),
        "pack8": nc.dram_tensor("pack8", [1, 1280], FP8, kind="ExternalInput").ap(),
        "y": nc.dram_tensor("y", [SLOTS, 512], F32, kind="ExternalOutput").ap(),
    }
    with tile.TileContext(nc) as tc:
        _build_kernel_body(tc, aps, groups)
    nc.finalize()
    _CACHE["key"] = plan_key
    _CACHE["nc"] = nc
    _CACHE["aps"] = aps
    return nc, aps


def _make_in_maps(hidden_states, Wq, bq, Wk, bk, lengths, batch_of, K, groups):
    hidden = np.asarray(hidden_states, dtype=np.float32)
    Wq = np.asarray(Wq, dtype=np.float32)
    Wk = np.asarray(Wk, dtype=np.float32)
    bqv = np.asarray(bq, dtype=np.float32)
    bkv = np.asarray(bk, dtype=np.float32)
    lens = np.asarray(lengths).astype(np.int64)

    packb = np.zeros((128, PB), dtype=BF16NP)
    p = np.arange(128)
    packb[:, OB_WK : OB_WK + 2048] = (
        np.ascontiguousarray(Wk.T).reshape(JC, 128, H).transpose(1, 0, 2).reshape(128, 2048)
    ).astype(BF16NP)
    packq = (
        (np.ascontiguousarray(Wq.T) / S)
        .reshape(JC, 128, H)
        .transpose(1, 0, 2)
        .reshape(128, 2048)
    ).astype(BF16NP)
    sel = np.zeros((128, 512), dtype=BF16NP)
    for g in range(SLOTS):
        sel[g, g * 128 : (g + 1) * 128] = BF16NP(1.0)
    packb[:, OB_SEL : OB_SEL + 512] = sel
    for g in range(SLOTS):
        packb[:, OB_IND + g * SLOTS + g] = BF16NP(1.0)
    packb[0, OB_ONESR : OB_ONESR + 128] = BF16NP(1.0)
    packb[:, OB_ONESC] = BF16NP(1.0)
    packb[0, OB_BK : OB_BK + 512] = bkv.astype(BF16NP)

    base_packf = np.zeros((128, PF), dtype=np.float32)
    base_packf[0:SLOTS, OF_BQ : OF_BQ + 512] = bqv[None, :]

    pack8 = np.zeros((1, 1280), dtype=FP8NP)
    # two-term fp8 split with a scaled residual: k-tile 0 = 1 * fp8(bk),
    # k-tile 1 = (1/16) * fp8(16*(bk - fp8(bk))), reproducing bk to ~1e-5.
    pack8[0, 0:128] = FP8NP(1.0)
    pack8[0, 128:256] = FP8NP(1.0 / 16.0)
    bk8 = bkv.astype(FP8NP)
    pack8[0, 256 : 256 + 512] = bk8
    pack8[0, 768 : 768 + 512] = (
        16.0 * (bkv - bk8.astype(np.float32))
    ).astype(FP8NP)

    in_maps = []
    for c in range(NCORES):
        hs = hidden[:, batch_of[c], :]  # [S, 4, H]
        xh = (
            hs.transpose(2, 1, 0).reshape(JC, 128, SLOTS, S).reshape(JC, 128, TOK_CORE)
        ).astype(BF16NP)
        packf = base_packf.copy()
        zi = 0
        for s, hh, nz in groups:
            ln = int(lens[batch_of[c, s]])
            for t in range(nz):
                s0 = hh * CHUNK + t * 128
                valid = (s0 + p) < ln
                packf[:, OF_MASK + zi] = np.where(valid, 0.0, MASK_NEG)
                zi += 1
        in_maps.append(
            {
                "xh": np.ascontiguousarray(xh),
                "packb": packb,
                "packq": packq,
                "packf": packf,
                "pack8": pack8,
            }
        )
    return in_maps


def run(hidden_states, Wq, bq, Wk, bk, lengths, trace=False):
    batch_of, K, groups = _plan(lengths)
    nc, _ = _get_program(tuple(groups))
    in_maps = _make_in_maps(
        hidden_states, Wq, bq, Wk, bk, lengths, batch_of, K, groups
    )
    res = run_bass_kernel_spmd(nc, in_maps, core_ids=list(range(NCORES)), trace=trace)
    out = np.zeros((B, H), dtype=np.float32)
    for c in range(NCORES):
        yc = np.asarray(res.results[c]["y"], dtype=np.float32)
        for s in range(SLOTS):
            out[batch_of[c, s]] = yc[s]
    return out, res


def kernel(hidden_states, Wq, bq, Wk, bk, lengths):
    out, _ = run(hidden_states, Wq, bq, Wk, bk, lengths)
    return out


# revision 26
# speedup vs baseline: 1.0545x; 1.0305x over previous
"""Trainium2 Bass kernel for nn_Attention_82660940579436 (v2).

Computation (see reference):
    q     = mean_s(hidden @ Wq.T + bq)            [B, H]
    key   = tanh(hidden @ Wk.T + bk)              [S, B, H]
    score = einsum('bsh,bh->bs', key, q) + mask   [B, S]
    out   = softmax(score) @ key                  [B, H]

Key observations driving this version:
  * Tokens with s >= lengths[b] get softmax weight exactly 0, so keys /
    scores / weighted sums are only needed for s < lengths[b] (a PREFIX of
    each batch's tokens).  Only the q-mean needs every token.
  * The host can pre-transpose + pre-cast hidden to bf16 "hT" layout
    [jc, j, tok] so the device does ZERO transposes: the z matmul consumes
    hT chunks as the stationary operand directly from DMA.
  * Batches are assigned to (core, slot) so that each slot's max length
    (over cores) is small: sort lengths desc, slot s takes ranks [8s, 8s+8).
    All cores then run the SAME program shape (z-tile counts per slot are
    global maxima); per-core data (hT, masks) differs.

Device program per core (4 slots x 4096 tokens; z-tiles of 128 tokens):
  Phase A, per 2048-token chunk-group (8 groups, z-rich first):
    - 4 HWDGE DMAs load hT chunks [128 j, 2048 tok] bf16 (one per j-chunk)
    - mean: per chunk, fold-tree (DVE) or Copy+accum (ACT) -> csum [128,1];
      m[j, (jc,g)] = csum_h0 + csum_h1 (bf16)
    - per z-tile: PE bias matmul (ones x bk) + 4 z matmuls (hT chunk
      stationary, WkT moving) -> PSUM [128 tok, 512]; ACT tanh -> keys bf16
  q block (emitted mid z-stream so PE reaches it right as the mean lands):
    q = m @ (WqT/S) (PE, bf16) ; q += bq (DVE, reads PSUM) ;
    qrep_g = sel_g.T @ q (PE) -> SBUF bf16 [128, 512] per slot
  Phase B, per z-tile:
    prod = keys[t] * qrep_slot   (DVE 2x / Pool split)
    score = rowsum(prod)         (DVE fold-tree / ACT accum split)
    e = exp(score + mask)        (ACT; mask -60 for invalid tokens)
    ei = ind_slot * e            (DVE tensor_scalar [128,4] bf16)
    numer += ei.T @ keys[t] ; den += ei.T @ ones   (PE, PSUM accumulate)
  out = numer / den -> DMA

Cost-model notes (TimelineSim/InstructionCostModel is the graded metric):
  matmul = out_free x 0.4167ns (bf16, warm); DMA = desc/16 x elem/22.5 (2x
  penalty below 512B runs -- hence 2048-token bf16 chunk rows); DVE
  TensorTensor bf16 SBUF = 2x mode; TensorReduce = 1x; ACT = 1/cycle
  + ~185ns init, accum_out +187ns.  fp8 DoubleRow would halve PE but
  measures 3.9e-2 rel err (> 2e-2 gate) -- rejected.
"""

import sys

import numpy as np

if "/opt/trn_rl_repo" not in sys.path:
    sys.path.append("/opt/trn_rl_repo")

import ml_dtypes  # noqa: E402

FP8NP = ml_dtypes.float8_e4m3fn

import concourse.bacc as bacc  # noqa: E402
import concourse.mybir as mybir  # noqa: E402
import concourse.tile as tile  # noqa: E402
from concourse.bass_utils import run_bass_kernel_spmd  # noqa: E402

S, B, H = 4096, 32, 512
NCORES = 8
SLOTS = 4  # batches per core
SLOT_TOK = S  # tokens per slot
CHUNK = 2048  # tokens per DMA chunk (4KB bf16 rows: no <512B DMA penalty)
JC = H // 128  # 4 j-chunks
TOK_CORE = SLOTS * SLOT_TOK

F32 = mybir.dt.float32
BF16 = mybir.dt.bfloat16
FP8 = mybir.dt.float8e4
AF = mybir.ActivationFunctionType
ALU = mybir.AluOpType
BF16NP = ml_dtypes.bfloat16
MASK_NEG = -60.0

# bf16 const pack offsets (elements); WqT/S ships separately (packq) so the
# startup-critical const DMA stays small.
OB_WK = 0  # [128, 4*512] WkT chunks
OB_SEL = 2048  # [4, 4*128] qrep selectors
OB_IND = 2560  # [128, 4*4] slot indicators
OB_ONESR = 2576  # [1, 128] ones row
OB_ONESC = 2704  # [128, 1] ones col
OB_BK = 2705  # [1, 512] bk
PB = 3217
# f32 const pack offsets
PF_PAD = 80  # mask columns (>= NZ)
OF_MASK = 0  # [128, PF_PAD]
OF_ZERO = PF_PAD  # [128, 1]
OF_BQ = PF_PAD + 1  # [4, 512] bq rows
PF = PF_PAD + 1 + 512

KNOBS = {
    "zps_bufs": 2,
    "fullz": 4,  # z-groups loaded full via the 2-buf xf pool (serve mean too)
    "q_after": 35,  # emit q block after this many z-tiles (min: first 2 groups)
    "b_catch": 4,  # phase-B tiles advanced per z-tile once past QI
    "b_stagger": 3,  # numer/TSP trail the mul/fold front by this many tiles
    "mul_pool_mod": 2,  # z-tile zi uses Pool mul when zi % mod == mod-1
    "red_act_mod": 3,  # z-tile zi reduces via ACT accum when zi % mod == 1
}

import json as _json
import os as _os

if _os.environ.get("KERNEL_KNOBS"):
    KNOBS.update(_json.loads(_os.environ["KERNEL_KNOBS"]))


def _plan(lengths):
    lens = np.asarray(lengths).astype(np.int64)
    order = np.argsort(-lens, kind="stable")
    batch_of = np.zeros((NCORES, SLOTS), dtype=np.int64)
    for s in range(SLOTS):
        for c in range(NCORES):
            batch_of[c, s] = order[NCORES * s + c]
    K = []
    for s in range(SLOTS):
        mx = int(lens[order[NCORES * s : NCORES * (s + 1)]].max())
        K.append(min(32, -(-mx // 128)))
    groups = []  # (slot, half, nz)
    for s in range(SLOTS):
        for hh in range(2):
            nz = max(0, min(16, K[s] - 16 * hh))
            groups.append((s, hh, nz))
    groups.sort(key=lambda x: (-x[2], x[0], x[1]))
    return batch_of, K, groups


def _build_kernel_body(tc, aps, groups):
    nc = tc.nc
    xh, packb, packf, y = aps["xh"], aps["packb"], aps["packf"], aps["y"]
    NZ = sum(g[2] for g in groups)

    zgroups = [g for g in groups if g[2] > 0]  # z-order (nz desc)
    mgroups = [g for g in groups if g[2] == 0]  # mean-only
    NFULL = min(KNOBS["fullz"], len(zgroups))
    fullz = zgroups[:NFULL]
    trimz = zgroups[NFULL:]
    dls = mgroups + trimz  # groups whose full chunk loads via the dl pool

    from contextlib import ExitStack

    with ExitStack() as ctx:
        consts = ctx.enter_context(tc.tile_pool(name="consts", bufs=1))
        pxf = ctx.enter_context(tc.tile_pool(name="xf", bufs=2))
        pdl = ctx.enter_context(tc.tile_pool(name="dl", bufs=2))
        ptz = ctx.enter_context(tc.tile_pool(name="tz", bufs=1))
        pkeys = ctx.enter_context(tc.tile_pool(name="keys", bufs=max(NZ, 1)))
        pfold = ctx.enter_context(tc.tile_pool(name="fold", bufs=4))
        pascr = ctx.enter_context(tc.tile_pool(name="ascr", bufs=2))
        pprod = ctx.enter_context(tc.tile_pool(name="prod", bufs=6))
        psmall = ctx.enter_context(tc.tile_pool(name="small", bufs=10))
        pacc = ctx.enter_context(tc.tile_pool(name="acc", bufs=1))
        ps_z = ctx.enter_context(
            tc.tile_pool(name="ps_z", bufs=KNOBS["zps_bufs"], space="PSUM")
        )
        ps_q = ctx.enter_context(tc.tile_pool(name="ps_q", bufs=1, space="PSUM"))
        ps_qr = ctx.enter_context(tc.tile_pool(name="ps_qr", bufs=2, space="PSUM"))
        ps_acc = ctx.enter_context(tc.tile_pool(name="ps_acc", bufs=1, space="PSUM"))

        cb = consts.tile([128, PB], BF16)
        # small consts (bk/ones/ind/sel) land in ~1us; WK chunks follow
        # interleaved with group 0's loads so the PE starts at ~3us.
        nc.sync.dma_start(cb[:, 2048:PB], packb[:, 2048:PB])
        cf = consts.tile([128, PF], F32)
        cq = consts.tile([128, 2048], BF16)  # WqT/S; DMA deferred
        c8 = consts.tile([1, 1280], FP8)
        nc.sync.dma_start(c8, aps["pack8"])
        ones8_dr = c8[0:1, 0:256].rearrange("p (two f) -> p two f", two=2)
        bk8_dr = c8[0:1, 256:1280].rearrange("p (two f) -> p two f", two=2)

        def wk_sb(c):
            return cb[:, OB_WK + c * 512 : OB_WK + (c + 1) * 512]

        def wq_sb(c):
            return cq[:, c * 512 : (c + 1) * 512]

        def sel_sb(g):
            return cb[0:SLOTS, OB_SEL + g * 128 : OB_SEL + (g + 1) * 128]

        def ind_sb(g):
            return cb[:, OB_IND + g * SLOTS : OB_IND + (g + 1) * SLOTS]

        ones_row = cb[0:1, OB_ONESR : OB_ONESR + 128]
        ones_col = cb[:, OB_ONESC : OB_ONESC + 1]
        bk_row = cb[0:1, OB_BK : OB_BK + 512]
        mask_sb = cf[:, OF_MASK : OF_MASK + PF_PAD]
        zero_sb = cf[:, OF_ZERO : OF_ZERO + 1]
        bq_sb = cf[0:SLOTS, OF_BQ : OF_BQ + 512]

        m_sb = pacc.tile([128, SLOTS * JC], BF16)  # col = jc*4 + g
        mparts = pacc.tile([128, 2 * SLOTS * JC], F32)  # col = (jc*4+g)*2 + half

        # ---------------- emission helpers ----------------
        def emit_mean(xt, s, hh):
            """chunk tiles -> csum [128,1] per jc, into mparts."""
            for jc in range(JC):
                dst = mparts[
                    :, (jc * SLOTS + s) * 2 + hh : (jc * SLOTS + s) * 2 + hh + 1
                ]
                f = pfold.tile([128, 1024], BF16, tag="fold")
                nc.vector.tensor_add(f, xt[jc][:, 0:1024], xt[jc][:, 1024:2048])
                nc.vector.tensor_add(f[:, 0:512], f[:, 0:512], f[:, 512:1024])
                nc.vector.tensor_add(f[:, 0:256], f[:, 0:256], f[:, 256:512])
                nc.vector.tensor_add(f[:, 0:128], f[:, 0:128], f[:, 128:256])
                nc.vector.tensor_reduce(
                    dst, f[:, 0:128], axis=mybir.AxisListType.X, op=ALU.add
                )

        def load_full(s, hh):
            base = s * SLOT_TOK + hh * CHUNK
            xt = []
            for jc in range(JC):
                t = pxf.tile([128, CHUNK], BF16, tag=f"xf{jc}")
                nc.sync.dma_start(t, xh[jc, :, base : base + CHUNK])
                xt.append(t)
            emit_mean(xt, s, hh)
            return xt

        def load_dl(s, hh):
            base = s * SLOT_TOK + hh * CHUNK
            xt = []
            for jc in range(JC):
                t = pdl.tile([128, CHUNK], BF16, tag=f"dl{jc}")
                nc.sync.dma_start(t, xh[jc, :, base : base + CHUNK])
                xt.append(t)
            emit_mean(xt, s, hh)

        def load_trim(idx, s, hh, nz):
            base = s * SLOT_TOK + hh * CHUNK
            w = nz * 128
            xt = []
            for jc in range(JC):
                t = ptz.tile([128, w], BF16, tag=f"tz{idx}_{jc}")
                nc.sync.dma_start(t, xh[jc, :, base : base + w])
                xt.append(t)
            return xt

        def emit_madds():
            for col in range(SLOTS * JC):
                nc.vector.tensor_add(
                    m_sb[:, col : col + 1],
                    mparts[:, 2 * col : 2 * col + 1],
                    mparts[:, 2 * col + 1 : 2 * col + 2],
                )

        qreps = []

        def emit_q_block():
            q_ps = ps_q.tile([SLOTS, 512], F32, tag="q")
            for jc in range(JC):
                nc.tensor.matmul(
                    q_ps,
                    m_sb[:, jc * SLOTS : (jc + 1) * SLOTS],
                    wq_sb(jc),
                    start=(jc == 0),
                    stop=(jc == JC - 1),
                )
            q_sbt = pacc.tile([SLOTS, 512], BF16)
            nc.vector.tensor_add(q_sbt, q_ps, bq_sb)
            for g in range(SLOTS):
                qr_ps = ps_qr.tile([128, 512], F32, tag="qr")
                nc.tensor.matmul(qr_ps, sel_sb(g), q_sbt, start=True, stop=True)
                qr = pacc.tile([128, 512], BF16, tag=f"qrep{g}")
                nc.vector.tensor_copy(qr, qr_ps)
                qreps.append(qr)

        # ---------------- phase B emitters (front/back stagger) ----------------
        numer = ps_acc.tile([SLOTS, 512], F32, tag="numer")
        den = ps_acc.tile([SLOTS, 1], F32, tag="den")
        keys = []
        zslot = []
        e_tiles = []

        def emit_front(zi):
            kt = keys[zi]
            s = zslot[zi]
            prod = pprod.tile([128, 512], BF16, tag="prod")
            if zi % KNOBS["mul_pool_mod"] == KNOBS["mul_pool_mod"] - 1:
                nc.gpsimd.tensor_mul(prod, kt, qreps[s])
            else:
                nc.vector.tensor_mul(prod, kt, qreps[s])
            sc = psmall.tile([128, 1], F32, tag="sc")
            ram = KNOBS["red_act_mod"]
            if ram and zi % ram == 1:
                scr = pascr.tile([128, 512], BF16, tag="bscr")
                nc.scalar.activation(scr, prod, AF.Copy, accum_out=sc)
            else:
                nc.vector.tensor_add(prod[:, 0:256], prod[:, 0:256], prod[:, 256:512])
                nc.vector.tensor_add(prod[:, 0:128], prod[:, 0:128], prod[:, 128:256])
                nc.vector.tensor_reduce(
                    sc, prod[:, 0:128], axis=mybir.AxisListType.X, op=ALU.add
                )
            e_t = psmall.tile([128, 1], F32, tag="e")
            nc.scalar.activation(e_t, sc, AF.Exp, bias=mask_sb[:, zi : zi + 1])
            e_tiles.append(e_t)

        def emit_back(zi):
            kt = keys[zi]
            ei = psmall.tile([128, SLOTS], BF16, tag="ei")
            nc.vector.tensor_scalar_mul(ei, ind_sb(zslot[zi]), e_tiles[zi])
            nc.tensor.matmul(numer, ei, kt, start=(zi == 0), stop=(zi == NZ - 1))
            nc.tensor.matmul(den, ei, ones_col, start=(zi == 0), stop=(zi == NZ - 1))

        # ---------------- the merged A/B schedule ----------------
        # DMA issue order (SP queue is FIFO): full z-groups interleaved with
        # dl (mean-copy) loads so every mean source has landed by ~40us while
        # the PE never waits for its next z chunk.
        ztile_plan = []  # (xt, local t, slot)

        def plan_group(xt, s, nz):
            for t in range(nz):
                ztile_plan.append((xt, t, s))

        # batch0 inline: WK slices interleaved with group 0's chunks so the
        # PE's first z matmuls start as early as possible
        nst = 0
        s0, h0 = fullz[0][:2]
        base0 = s0 * SLOT_TOK + h0 * CHUNK
        xt0 = []
        for jc in range(JC):
            nc.sync.dma_start(
                cb[:, OB_WK + jc * 512 : OB_WK + (jc + 1) * 512],
                packb[:, OB_WK + jc * 512 : OB_WK + (jc + 1) * 512],
            )
            t0 = pxf.tile([128, CHUNK], BF16, tag=f"xf{jc}")
            nc.sync.dma_start(t0, xh[jc, :, base0 : base0 + CHUNK])
            xt0.append(t0)
        emit_mean(xt0, s0, h0)
        nc.sync.dma_start(cf, packf)
        plan_group(xt0, fullz[0][0], fullz[0][2])
        if NFULL > 1:
            xt1 = load_full(*fullz[1][:2])
            plan_group(xt1, fullz[1][0], fullz[1][2])
        for d in dls[0:2]:
            load_dl(d[0], d[1])
        # batch1/2 described as thunks, emitted at group boundaries
        def emit_batch1():
            if NFULL > 2:
                xt = load_full(*fullz[2][:2])
                plan_group(xt, fullz[2][0], fullz[2][2])
            for d in dls[2:4]:
                load_dl(d[0], d[1])

        def emit_batch2():
            if NFULL > 3:
                xt = load_full(*fullz[3][:2])
                plan_group(xt, fullz[3][0], fullz[3][2])
            nc.sync.dma_start(cq, aps["packq"])
            for d in dls[4:]:
                load_dl(d[0], d[1])
            for i, (s, hh, nz) in enumerate(trimz):
                xt = load_trim(i, s, hh, nz)
                plan_group(xt, s, nz)
            emit_madds()

        tiles01 = nst + fullz[0][2] + (fullz[1][2] if NFULL > 1 else 0)
        QI = max(min(KNOBS["q_after"], NZ - 1), min(tiles01 + 1, NZ - 1))
        bnd1 = nst + fullz[0][2]  # after group 0's tiles
        bnd2 = tiles01  # after group 1's tiles

        zi = 0
        fj = 0  # phase B front progress

        def emit_ztile(xt, t, s):
            zp = ps_z.tile([128, 512], F32, tag="z")
            # fp8 DoubleRow rank-2 bias: out = sum_i ones8[:,i,:].T @ bk8[:,i,:]
            # = bk broadcast over tokens, at 0.5 cycles/row (half the bf16 cost)
            nc.tensor.matmul(
                zp,
                ones8_dr,
                bk8_dr,
                start=True,
                stop=False,
                perf_mode=mybir.MatmulPerfMode.DoubleRow,
            )
            for jc in range(JC):
                nc.tensor.matmul(
                    zp,
                    xt[jc][:, t * 128 : (t + 1) * 128],
                    wk_sb(jc),
                    start=False,
                    stop=(jc == JC - 1),
                )
            kt = pkeys.tile([128, 512], BF16, tag="key")
            nc.scalar.activation(kt, zp, AF.Tanh, bias=zero_sb)
            keys.append(kt)
            zslot.append(s)

        while zi < NZ or fj < NZ:
            if zi < len(ztile_plan):
                emit_ztile(*ztile_plan[zi])
                zi += 1
                if zi == bnd1:
                    emit_batch1()
                if zi == bnd2:
                    emit_batch2()
                if zi == QI:
                    emit_q_block()
                if zi <= QI:
                    continue
            elif zi < NZ:
                raise RuntimeError("ztile_plan shorter than NZ")
            # advance phase B (front zi-stagger keeps DVE queue un-blocked)
            budget = KNOBS["b_catch"] if zi < NZ else NZ
            stag = KNOBS["b_stagger"]
            while budget > 0 and fj < NZ and (fj <= zi - 2 or zi >= NZ):
                emit_front(fj)
                if fj >= stag:
                    emit_back(fj - stag)
                fj += 1
                budget -= 1
            if zi >= NZ and fj >= NZ:
                break
        for r in range(max(NZ - KNOBS["b_stagger"], 0), NZ):
            emit_back(r)

        rcp = pacc.tile([SLOTS, 1], F32)
        nc.vector.reciprocal(rcp, den)
        out_sb = pacc.tile([SLOTS, 512], F32)
        nc.vector.tensor_scalar_mul(out_sb, numer, rcp)
        nc.sync.dma_start(y, out_sb)


_CACHE = {}


def _get_program(plan_key=None):
    if plan_key is None:
        return _CACHE["nc"], _CACHE["aps"]
    if _CACHE.get("key") == plan_key:
        return _CACHE["nc"], _CACHE["aps"]
    groups = list(plan_key)
    nc = bacc.Bacc(None, target_bir_lowering=False, debug=False)
    aps = {
        "xh": nc.dram_tensor("xh", [JC, 128, TOK_CORE], BF16, kind="ExternalInput").ap(),
        "packb": nc.dram_tensor("packb", [128, PB], BF16, kind="ExternalInput").ap(),
        "packq": nc.dram_tensor("packq", [128, 2048], BF16, kind="ExternalInput").ap(),
        "packf": nc.dram_tensor("packf", [128, PF], F32, kind="ExternalInput").ap(),
        "pack8": nc.dram_tensor("pack8", [1, 1280], FP8, kind="ExternalInput").ap(),
        "y": nc.dram_tensor("y", [SLOTS, 512], F32, kind="ExternalOutput").ap(),
    }
    with tile.TileContext(nc) as tc:
        _build_kernel_body(tc, aps, groups)
    nc.finalize()
    _CACHE["key"] = plan_key
    _CACHE["nc"] = nc
    _CACHE["aps"] = aps
    return nc, aps


def _make_in_maps(hidden_states, Wq, bq, Wk, bk, lengths, batch_of, K, groups):
    hidden = np.asarray(hidden_states, dtype=np.float32)
    Wq = np.asarray(Wq, dtype=np.float32)
    Wk = np.asarray(Wk, dtype=np.float32)
    bqv = np.asarray(bq, dtype=np.float32)
    bkv = np.asarray(bk, dtype=np.float32)
    lens = np.asarray(lengths).astype(np.int64)

    packb = np.zeros((128, PB), dtype=BF16NP)
    p = np.arange(128)
    packb[:, OB_WK : OB_WK + 2048] = (
        np.ascontiguousarray(Wk.T).reshape(JC, 128, H).transpose(1, 0, 2).reshape(128, 2048)
    ).astype(BF16NP)
    packq = (
        (np.ascontiguousarray(Wq.T) / S)
        .reshape(JC, 128, H)
        .transpose(1, 0, 2)
        .reshape(128, 2048)
    ).astype(BF16NP)
    sel = np.zeros((128, 512), dtype=BF16NP)
    for g in range(SLOTS):
        sel[g, g * 128 : (g + 1) * 128] = BF16NP(1.0)
    packb[:, OB_SEL : OB_SEL + 512] = sel
    for g in range(SLOTS):
        packb[:, OB_IND + g * SLOTS + g] = BF16NP(1.0)
    packb[0, OB_ONESR : OB_ONESR + 128] = BF16NP(1.0)
    packb[:, OB_ONESC] = BF16NP(1.0)
    packb[0, OB_BK : OB_BK + 512] = bkv.astype(BF16NP)

    base_packf = np.zeros((128, PF), dtype=np.float32)
    base_packf[0:SLOTS, OF_BQ : OF_BQ + 512] = bqv[None, :]

    pack8 = np.zeros((1, 1280), dtype=FP8NP)
    # two-term fp8 split with a scaled residual: k-tile 0 = 1 * fp8(bk),
    # k-tile 1 = (1/16) * fp8(16*(bk - fp8(bk))), reproducing bk to ~1e-5.
    pack8[0, 0:128] = FP8NP(1.0)
    pack8[0, 128:256] = FP8NP(1.0 / 16.0)
    bk8 = bkv.astype(FP8NP)
    pack8[0, 256 : 256 + 512] = bk8
    pack8[0, 768 : 768 + 512] = (
        16.0 * (bkv - bk8.astype(np.float32))
    ).astype(FP8NP)

    in_maps = []
    for c in range(NCORES):
        hs = hidden[:, batch_of[c], :]  # [S, 4, H]
        xh = (
            hs.transpose(2, 1, 0).reshape(JC, 128, SLOTS, S).reshape(JC, 128, TOK_CORE)
        ).astype(BF16NP)
        packf = base_packf.copy()
        zi = 0
        for s, hh, nz in groups:
            ln = int(lens[batch_of[c, s]])
            for t in range(nz):
                s0 = hh * CHUNK + t * 128
                valid = (s0 + p) < ln
                packf[:, OF_MASK + zi] = np.where(valid, 0.0, MASK_NEG)
                zi += 1
        in_maps.append(
            {
                "xh": np.ascontiguousarray(xh),
                "packb": packb,
                "packq": packq,
                "packf": packf,
                "pack8": pack8,
            }
        )
    return in_maps


def run(hidden_states, Wq, bq, Wk, bk, lengths, trace=False):
    batch_of, K, groups = _plan(lengths)
    nc, _ = _get_program(tuple(groups))
    in_maps = _make_in_maps(
        hidden_states, Wq, bq, Wk, bk, lengths, batch_of, K, groups
    )
    res = run_bass_kernel_spmd(nc, in_maps, core_ids=list(range(NCORES)), trace=trace)
    out = np.zeros((B, H), dtype=np.float32)
    for c in range(NCORES):
        yc = np.asarray(res.results[c]["y"], dtype=np.float32)
        for s in range(SLOTS):
            out[batch_of[c, s]] = yc[s]
    return out, res


def kernel(hidden_states, Wq, bq, Wk, bk, lengths):
    out, _ = run(hidden_states, Wq, bq, Wk, bk, lengths)
    return out
